# revision 6
# baseline (speedup 1.0000x reference)
# Bass/Tile TRN2 kernel for nn_ConvTSP (gated GCN, 3 layers + edge MLP).
#
# Sharding: 8 cores; core c owns node-rows [25c, 25c+25) of EVERY batch
# (4 sections x 25 rows x 200 cols of the (B,N,N,H) edge tensor).  This
# makes the per-batch mask lengths identical across cores, so the prefix
# masks can be baked into the (shared SPMD) program as static slice
# bounds; all remaining per-core masking enters through small input
# tensors.  The edge tensor stays SBUF-resident across all layers
# (layout [H=128 partitions, 20000 edge columns]); BatchNorm statistics
# are combined with one 8-core AllReduce per layer and the node-feature
# halves with one AllGather per layer.
import os
import numpy as np

B, N, H, L = 4, 200, 128, 3
HH = H // 2
NCORES = 8
RPC = N // NCORES          # 25 rows per core per batch
ROWS = B * RPC             # 100 local rows
COLS = ROWS * N            # 20000 local edge columns
SLAB = 5                   # rows per psum slab
NSLAB_SEC = RPC // SLAB    # 5 slabs per section
EPS_BN = 1e-5

_prog_cache = {}


def _build(lens, cntM, cntX):
    import concourse.bass as bass
    import concourse.bacc as bacc
    import concourse.tile as tile
    import concourse.mybir as mybir

    f32 = mybir.dt.float32
    bf16 = mybir.dt.bfloat16
    A = mybir.AluOpType
    AF = mybir.ActivationFunctionType
    X = mybir.AxisListType.X

    nc = bacc.Bacc(
        "TRN2", target_bir_lowering=False, debug=False,
        enable_asserts=False, num_devices=NCORES,
    )

    # ---- external inputs (per core) ----
    d_f4 = nc.dram_tensor("f4", [4, COLS], f32, kind="ExternalInput")
    d_xcf = nc.dram_tensor("xcf", [2, B * N], f32, kind="ExternalInput")
    d_xco = nc.dram_tensor("xco", [2, ROWS], f32, kind="ExternalInput")
    d_mi = nc.dram_tensor("mi", [128, ROWS], f32, kind="ExternalInput")
    d_mic = nc.dram_tensor("mic", [RPC, B], f32, kind="ExternalInput")
    d_wm = nc.dram_tensor("wm", [128, 3 * 512 + 128 + 2], f32, kind="ExternalInput")
    d_vb = nc.dram_tensor("vb", [128, 7 * L + 1], f32, kind="ExternalInput")
    d_vf = nc.dram_tensor("vf", [RPC, 2 * L * 128], f32, kind="ExternalInput")
    d_i25 = nc.dram_tensor("i25", [RPC, RPC], bf16, kind="ExternalInput")
    d_ones = nc.dram_tensor("ones1", [1, 512], bf16, kind="ExternalInput")
    d_mvb = nc.dram_tensor("mvb", [1, 2], f32, kind="ExternalInput")
    d_w0 = nc.dram_tensor("w0", [4, 128], f32, kind="ExternalInput")
    d_wco = nc.dram_tensor("wco", [2, 128], f32, kind="ExternalInput")
    d_y = nc.dram_tensor("y", [2, COLS], f32, kind="ExternalOutput")

    with tile.TileContext(nc) as tc:
        with (
            tc.tile_pool(name="pers", bufs=1) as pers,
            tc.tile_pool(name="ring", bufs=3) as ring,
            tc.tile_pool(name="gatep", bufs=3) as gatep,
            tc.tile_pool(name="gmp", bufs=3) as gmp,
            tc.tile_pool(name="sqp", bufs=3) as sqp,
            tc.tile_pool(name="rowp", bufs=3) as rowp,
            tc.tile_pool(name="htp", bufs=3) as htp,
            tc.tile_pool(name="smallp", bufs=4) as smallp,
            tc.tile_pool(name="rbp", bufs=3) as rbp,
            tc.tile_pool(name="psb", bufs=2, space="PSUM") as psb_pool,
            tc.tile_pool(name="pss", bufs=2, space="PSUM") as pss_pool,
            tc.tile_pool(name="dram", bufs=2, space="DRAM") as dram,
        ):
            # ---- persistent SBUF ----
            E = pers.tile([128, COLS], f32)
            ET = pers.tile([128, COLS], bf16)
            XF = pers.tile([128, B * N], f32)
            XO = pers.tile([128, ROWS], f32)
            VX = pers.tile([128, B * N], bf16)
            WM = pers.tile([128, 3 * 512 + 128 + 2], f32)
            VB = pers.tile([128, 7 * L + 1], f32)
            VF = pers.tile([RPC, 2 * L * 128], f32)
            MI = pers.tile([128, ROWS], f32)
            MIC = pers.tile([RPC, B], f32)
            I25 = pers.tile([RPC, RPC], bf16)
            ONES = pers.tile([1, 512], bf16)
            MVB = pers.tile([1, 2], f32)
            W0 = pers.tile([4, 128], f32)
            WCO = pers.tile([2, 128], f32)
            XCF = pers.tile([2, B * N], f32)
            XCO = pers.tile([2, ROWS], f32)
            AGG = pers.tile([128, ROWS], f32)
            DEN = pers.tile([128, ROWS], f32)
            R1 = pers.tile([128, ROWS], f32)
            R2 = pers.tile([128, ROWS], f32)
            XT = pers.tile([128, ROWS], f32)
            UPD = pers.tile([128, ROWS], f32)
            ARS = pers.tile([128, 4], f32)
            ARO = pers.tile([128, 4], f32)
            AGS = pers.tile([128, NCORES * ROWS], f32)
            SC = pers.tile([128, 24], f32)
            AB = pers.tile([128, 2 * ROWS], f32)
            JK = pers.tile([128, ROWS], bf16)   # junk sink for accum-only ops
            RC = pers.tile([128, ROWS], f32)

            def sc(i):
                return SC[:, i:i + 1]

            # ---- load inputs ----
            nc.sync.dma_start(WM[:], d_wm.ap())
            nc.sync.dma_start(VB[:], d_vb.ap())
            nc.sync.dma_start(VF[:], d_vf.ap())
            nc.sync.dma_start(MI[:], d_mi.ap())
            nc.sync.dma_start(MIC[:], d_mic.ap())
            nc.sync.dma_start(I25[:], d_i25.ap())
            nc.sync.dma_start(ONES[:], d_ones.ap())
            nc.sync.dma_start(MVB[:], d_mvb.ap())
            nc.sync.dma_start(W0[:], d_w0.ap())
            nc.sync.dma_start(WCO[:], d_wco.ap())
            nc.sync.dma_start(XCF[:], d_xcf.ap())
            nc.sync.dma_start(XCO[:], d_xco.ap())

            # ---- x init: x0 = w_coord.T @ coords ----
            for b in range(B):
                ps = pss_pool.tile([128, 256], f32, tag="pss")
                nc.tensor.matmul(ps[:, 0:N], lhsT=WCO[:], rhs=XCF[:, b * N:(b + 1) * N],
                                 start=True, stop=True)
                nc.scalar.activation(XF[:, b * N:(b + 1) * N], ps[:, 0:N], AF.Copy)
            pso = pss_pool.tile([128, 256], f32, tag="pss")
            nc.tensor.matmul(pso[:, 0:ROWS], lhsT=WCO[:], rhs=XCO[:], start=True, stop=True)
            nc.scalar.activation(XO[:], pso[:, 0:ROWS], AF.Copy)

            # ---- E init: E0 = W0.T @ F4 ----
            for s in range(COLS // 1000):
                f4t = ring.tile([4, 1000], f32)
                nc.sync.dma_start(f4t[:], d_f4.ap()[:, s * 1000:(s + 1) * 1000])
                ps = psb_pool.tile([128, 2, 512], f32, tag="psb")
                for k in range(2):
                    nc.tensor.matmul(ps[:, k, 0:500], lhsT=W0[:],
                                     rhs=f4t[:, k * 500:(k + 1) * 500],
                                     start=True, stop=True)
                nc.scalar.activation(
                    E[:, s * 1000:(s + 1) * 1000].rearrange("p (k j) -> p k j", j=500),
                    ps[:, :, 0:500], AF.Copy)

            # =========================== layers ===========================
            for l in range(L):
                wo = l * 512
                eU = WM[:, wo:wo + 128]
                eV = WM[:, wo + 128:wo + 256]
                nU = WM[:, wo + 256:wo + 384]
                nV = WM[:, wo + 384:wo + 512]
                eVb = VB[:, 7 * l + 0:7 * l + 1]
                nUb = VB[:, 7 * l + 1:7 * l + 2]
                nVb = VB[:, 7 * l + 2:7 * l + 3]
                gE = VB[:, 7 * l + 3:7 * l + 4]
                bE = VB[:, 7 * l + 4:7 * l + 5]
                gN = VB[:, 7 * l + 5:7 * l + 6]
                bN = VB[:, 7 * l + 6:7 * l + 7]
                eVbF = VF[:, (2 * l) * 128:(2 * l + 1) * 128]
                eUbF = VF[:, (2 * l + 1) * 128:(2 * l + 2) * 128]

                # ---- per-section prep: rowbiasT and Vx ----
                rbs = []
                for b in range(B):
                    psR = pss_pool.tile([RPC, 256], f32, tag="pss")
                    nc.tensor.matmul(psR[:, 0:128], lhsT=XO[:, b * RPC:(b + 1) * RPC],
                                     rhs=eV, start=True, stop=True)
                    t1 = smallp.tile([RPC, 128], f32, tag="t1")
                    # t1 = (psR + eVbF) * maski_col
                    nc.vector.scalar_tensor_tensor(t1[:], in0=psR[:, 0:128], scalar=0.0,
                                                   in1=eVbF, op0=A.bypass, op1=A.add)
                    nc.vector.tensor_scalar(t1[:], in0=t1[:], scalar1=MIC[:, b:b + 1],
                                            scalar2=None, op0=A.mult)
                    rb = rbp.tile([RPC, 128], bf16)
                    nc.vector.tensor_tensor(rb[:], t1[:], eUbF, A.add)
                    rbs.append(rb)
                    # Vx_b = nV.T @ x_b + nVb   (bf16; only cols < len used)
                    psV = pss_pool.tile([128, 256], f32, tag="pss")
                    nc.tensor.matmul(psV[:, 0:N], lhsT=nV, rhs=XF[:, b * N:(b + 1) * N],
                                     start=True, stop=True)
                    nc.vector.tensor_scalar(VX[:, b * N:(b + 1) * N], in0=psV[:, 0:N],
                                            scalar1=nVb, scalar2=None, op0=A.add)

                # ---- slab loop ----
                for b in range(B):
                    lb = lens[b]
                    for ss in range(NSLAB_SEC):
                        r0 = b * RPC + ss * SLAB          # local row idx of slab start
                        c0 = r0 * N
                        ps = psb_pool.tile([128, SLAB, 256], f32, tag="psb")
                        for r in range(SLAB):
                            cr = c0 + r * N
                            nc.tensor.matmul(ps[:, r, 0:N], lhsT=eU,
                                             rhs=E[:, cr:cr + N], start=True, stop=False)
                            nc.tensor.matmul(ps[:, r, 0:lb], lhsT=eV,
                                             rhs=XF[:, b * N:b * N + lb],
                                             start=False, stop=False)
                            nc.tensor.matmul(
                                ps[:, r, 0:N], lhsT=rbs[b][:],
                                rhs=I25[:, ss * SLAB + r:ss * SLAB + r + 1].broadcast_to((RPC, N)),
                                start=False, stop=True)
                        ets = ET[:, c0:c0 + SLAB * N].rearrange("p (r j) -> p r j", j=N)
                        nc.scalar.activation(ets, ps[:, :, 0:N], AF.Copy)
                        gt = gatep.tile([128, SLAB, N], bf16)
                        nc.scalar.activation(gt[:, :, 0:lb], ps[:, :, 0:lb], AF.Sigmoid,
                                             bias=eVb)
                        # stats first (so the stats AllReduce can launch early)
                        nc.vector.reduce_sum(R1[:, r0:r0 + SLAB], ets[:, :, 0:lb], axis=X)
                        sq = sqp.tile([128, SLAB, N], bf16)
                        nc.vector.scalar_tensor_tensor(sq[:, :, 0:lb], in0=ets[:, :, 0:lb],
                                                       scalar=0.0, in1=ets[:, :, 0:lb],
                                                       op0=A.bypass, op1=A.mult)
                        nc.vector.reduce_sum(R2[:, r0:r0 + SLAB], sq[:, :, 0:lb], axis=X)
                        # gate aggregation
                        gm = gmp.tile([128, SLAB, N], bf16)
                        nc.vector.tensor_tensor(
                            gm[:, :, 0:lb], gt[:, :, 0:lb],
                            VX[:, b * N:b * N + lb].unsqueeze(1).broadcast_to((128, SLAB, lb)),
                            A.mult)
                        nc.vector.reduce_sum(AGG[:, r0:r0 + SLAB], gm[:, :, 0:lb], axis=X)
                        nc.vector.reduce_sum(DEN[:, r0:r0 + SLAB], gt[:, :, 0:lb], axis=X)

                # ---- local stat totals ----
                nc.vector.scalar_tensor_tensor(JK[:], in0=R1[:], scalar=0.0, in1=MI[:],
                                               op0=A.bypass, op1=A.mult,
                                               accum_out=ARS[:, 0:1])
                nc.vector.scalar_tensor_tensor(JK[:], in0=R2[:], scalar=0.0, in1=MI[:],
                                               op0=A.bypass, op1=A.mult,
                                               accum_out=ARS[:, 1:2])

                # ---- x branch (own rows) ----
                nc.vector.tensor_scalar(RC[:], in0=DEN[:], scalar1=1e-20, scalar2=None,
                                        op0=A.add)
                nc.vector.reciprocal(RC[:], RC[:])
                for b in range(B):
                    sl = slice(b * RPC, (b + 1) * RPC)
                    psU = pss_pool.tile([128, 256], f32, tag="pss")
                    nc.tensor.matmul(psU[:, 0:RPC], lhsT=nU, rhs=XO[:, sl],
                                     start=True, stop=True)
                    tg = smallp.tile([128, RPC], f32, tag="tg")
                    nc.vector.tensor_tensor(tg[:], AGG[:, sl], RC[:, sl], A.mult)
                    nc.vector.scalar_tensor_tensor(XT[:, sl], in0=psU[:, 0:RPC],
                                                   scalar=nUb, in1=tg[:],
                                                   op0=A.add, op1=A.add)
                nc.vector.tensor_tensor(XT[:], XT[:], MI[:], A.mult)
                nc.vector.reduce_sum(ARS[:, 2:3], XT[:], axis=X)
                nc.vector.scalar_tensor_tensor(JK[:], in0=XT[:], scalar=0.0, in1=XT[:],
                                               op0=A.bypass, op1=A.mult,
                                               accum_out=ARS[:, 3:4])

                # ---- AllReduce of stats ----
                ar_in = dram.tile([128, 4], f32, tag="arin")
                ar_out = dram.tile([128, 4], f32, tag="arout", addr_space="Shared")
                nc.sync.dma_start(ar_in[:], ARS[:])
                nc.gpsimd.collective_compute(
                    "AllReduce", A.add, replica_groups=[list(range(NCORES))],
                    ins=[ar_in[:].opt()], outs=[ar_out[:].opt()])
                nc.sync.dma_start(ARO[:], ar_out[:])

                # ---- BN coefficient math (small, replicated on all cores) ----
                invM = 1.0 / cntM
                invX = 1.0 / cntX

                def heron_rsqrt(v_col, out_col, s1, s2):
                    # out = 1/sqrt(v) = sqrt(1/v):  x = 1/v;  Heron on x with
                    # y <- (y + x*(1/y))/2, seeded y = (1+x)/2 (globally convergent)
                    nc.vector.reciprocal(s1, v_col)                     # x
                    nc.vector.tensor_scalar(out_col, in0=s1, scalar1=1.0, scalar2=0.5,
                                            op0=A.add, op1=A.mult)      # y0
                    for _ in range(7):
                        nc.vector.reciprocal(s2, out_col)               # 1/y
                        nc.vector.tensor_tensor(s2, s1, s2, A.mult)     # x/y
                        nc.vector.tensor_tensor(s2, s2, out_col, A.add)
                        nc.vector.tensor_scalar(out_col, in0=s2, scalar1=0.5,
                                                scalar2=None, op0=A.mult)

                # e-stats: S1t = ARO0 + cntM*eVb ; S2t = ARO1 + 2 eVb ARO0 + cntM eVb^2
                nc.vector.tensor_scalar(sc(0), in0=eVb, scalar1=cntM, scalar2=None,
                                        op0=A.mult)
                nc.vector.tensor_tensor(sc(0), sc(0), ARO[:, 0:1], A.add)      # S1t
                nc.vector.tensor_tensor(sc(1), eVb, ARO[:, 0:1], A.mult)
                nc.vector.tensor_scalar(sc(1), in0=sc(1), scalar1=2.0, scalar2=None,
                                        op0=A.mult)
                nc.vector.tensor_tensor(sc(2), eVb, eVb, A.mult)
                nc.vector.tensor_scalar(sc(2), in0=sc(2), scalar1=cntM, scalar2=None,
                                        op0=A.mult)
                nc.vector.tensor_tensor(sc(1), sc(1), sc(2), A.add)
                nc.vector.tensor_tensor(sc(1), sc(1), ARO[:, 1:2], A.add)      # S2t
                nc.vector.tensor_scalar(sc(3), in0=sc(0), scalar1=invM, scalar2=None,
                                        op0=A.mult)                            # meanE
                nc.vector.tensor_tensor(sc(4), sc(3), sc(3), A.mult)
                nc.vector.tensor_scalar(sc(5), in0=sc(1), scalar1=invM, scalar2=None,
                                        op0=A.mult)
                nc.vector.tensor_tensor(sc(5), sc(5), sc(4), A.subtract)       # varE
                nc.vector.tensor_scalar(sc(5), in0=sc(5), scalar1=EPS_BN, scalar2=None,
                                        op0=A.add)
                heron_rsqrt(sc(5), sc(6), sc(7), sc(8))                        # invstdE
                nc.vector.tensor_tensor(sc(9), gE, sc(6), A.mult)              # scaleE
                nc.vector.tensor_tensor(sc(10), sc(3), sc(9), A.mult)
                nc.vector.tensor_tensor(sc(10), bE, sc(10), A.subtract)        # shiftE
                # A/B row coefficient tiles
                nc.vector.tensor_scalar(sc(11), in0=sc(9), scalar1=-1.0, scalar2=None,
                                        op0=A.add)                             # scaleE-1
                nc.vector.tensor_tensor(sc(12), sc(11), eVb, A.mult)
                nc.vector.tensor_tensor(sc(12), sc(10), sc(12), A.add)         # shift+(s-1)eVb
                nc.vector.tensor_scalar(AB[:, 0:ROWS], in0=MI[:], scalar1=sc(11),
                                        scalar2=1.0, op0=A.mult, op1=A.add)    # A
                nc.vector.tensor_scalar(AB[:, ROWS:2 * ROWS], in0=MI[:], scalar1=sc(12),
                                        scalar2=eVb, op0=A.mult, op1=A.add)    # Bc
                # x-stats
                nc.vector.tensor_scalar(sc(13), in0=ARO[:, 2:3], scalar1=invX,
                                        scalar2=None, op0=A.mult)              # meanX
                nc.vector.tensor_tensor(sc(14), sc(13), sc(13), A.mult)
                nc.vector.tensor_scalar(sc(15), in0=ARO[:, 3:4], scalar1=invX,
                                        scalar2=None, op0=A.mult)
                nc.vector.tensor_tensor(sc(15), sc(15), sc(14), A.subtract)    # varX
                nc.vector.tensor_scalar(sc(15), in0=sc(15), scalar1=EPS_BN, scalar2=None,
                                        op0=A.add)
                heron_rsqrt(sc(15), sc(16), sc(17), sc(18))
                nc.vector.tensor_tensor(sc(19), gN, sc(16), A.mult)            # scaleX
                nc.vector.tensor_tensor(sc(20), sc(13), sc(19), A.mult)
                nc.vector.tensor_tensor(sc(20), bN, sc(20), A.subtract)        # shiftX

                # ---- x apply + AllGather ----
                xr = smallp.tile([128, ROWS], f32, tag="xr")
                nc.scalar.activation(xr[:], XT[:], AF.Relu, bias=sc(20), scale=sc(19))
                nc.vector.tensor_tensor(UPD[:], xr[:], MI[:], A.mult)
                nc.vector.tensor_tensor(XO[:], XO[:], UPD[:], A.add)
                ag_in = dram.tile([128, ROWS], f32, tag="agin")
                ag_out = dram.tile([NCORES, 128, ROWS], f32, tag="agout",
                                   addr_space="Shared")
                nc.sync.dma_start(ag_in[:], UPD[:])
                nc.gpsimd.collective_compute(
                    "AllGather", A.bypass, replica_groups=[list(range(NCORES))],
                    ins=[ag_in[:].opt()], outs=[ag_out[:].opt()])
                nc.sync.dma_start(
                    AGS[:].rearrange("p (c k) -> p c k", k=ROWS),
                    ag_out[:].rearrange("c p k -> p c k"))

                # ---- e apply: E += relu(A*ET + Bc) on prefix, relu(ET) on suffix ----
                for b in range(B):
                    lb = lens[b]
                    for ss in range(NSLAB_SEC):
                        r0 = b * RPC + ss * SLAB
                        c0 = r0 * N
                        rt = rowp.tile([128, SLAB, N], bf16)
                        for r in range(SLAB):
                            rr = r0 + r
                            nc.vector.tensor_scalar(
                                rt[:, r, 0:lb], in0=ET[:, rr * N:rr * N + lb],
                                scalar1=AB[:, rr:rr + 1],
                                scalar2=AB[:, ROWS + rr:ROWS + rr + 1],
                                op0=A.mult, op1=A.add)
                        esl = E[:, c0:c0 + SLAB * N].rearrange("p (r j) -> p r j", j=N)
                        etsl = ET[:, c0:c0 + SLAB * N].rearrange("p (r j) -> p r j", j=N)
                        nc.vector.scalar_tensor_tensor(
                            esl[:, :, 0:lb], in0=rt[:, :, 0:lb], scalar=0.0,
                            in1=esl[:, :, 0:lb], op0=A.max, op1=A.add)
                        if lb < N:
                            nc.vector.scalar_tensor_tensor(
                                esl[:, :, lb:N], in0=etsl[:, :, lb:N], scalar=0.0,
                                in1=esl[:, :, lb:N], op0=A.max, op1=A.add)

                # ---- fold gathered x updates into x_full ----
                for b in range(B):
                    nc.vector.tensor_tensor(
                        XF[:, b * N:(b + 1) * N].rearrange("p (c k) -> p c k", k=RPC),
                        XF[:, b * N:(b + 1) * N].rearrange("p (c k) -> p c k", k=RPC),
                        AGS[:].rearrange("p (c q k) -> p c q k", q=B, k=RPC)[:, :, b, :],
                        A.add)

            # =========================== edge MLP ===========================
            mU = WM[:, 3 * 512:3 * 512 + 128]
            mUb = VB[:, 7 * L:7 * L + 1]
            mV16 = pers.tile([128, 2], bf16)
            nc.vector.tensor_copy(mV16[:], WM[:, 3 * 512 + 128:3 * 512 + 130])
            mvb16 = pers.tile([1, 2], bf16)
            nc.vector.tensor_copy(mvb16[:], MVB[:])
            for s in range(COLS // 1000):
                c0 = s * 1000
                psH = psb_pool.tile([128, 2, 512], f32, tag="psb")
                for k in range(2):
                    nc.tensor.matmul(psH[:, k, 0:500], lhsT=mU,
                                     rhs=E[:, c0 + k * 500:c0 + (k + 1) * 500],
                                     start=True, stop=True)
                ht = htp.tile([128, 2, 500], bf16)
                nc.scalar.activation(ht[:], psH[:, :, 0:500], AF.Relu, bias=mUb)
                psY = psb_pool.tile([2, 2, 512], f32, tag="psb")
                for k in range(2):
                    nc.tensor.matmul(psY[:, k, 0:500], lhsT=mV16[:], rhs=ht[:, k, :],
                                     start=True, stop=False)
                    nc.tensor.matmul(psY[:, k, 0:500], lhsT=mvb16[:],
                                     rhs=ONES[:, 0:500], start=False, stop=True)
                yst = htp.tile([2, 1000], f32, tag="yst")
                nc.scalar.activation(
                    yst[:].rearrange("p (k j) -> p k j", j=500),
                    psY[:, :, 0:500], AF.Copy)
                nc.sync.dma_start(d_y.ap()[:, c0:c0 + 1000], yst[:])

    nc.compile()
    return nc


def _prep_inputs(x_edges, x_edges_values, x_nodes_coord, mask,
                 w_coord, w_eval, emb_edge,
                 eU_w, eU_b, eV_w, eV_b, nU_w, nU_b, nV_w, nV_b,
                 bnE_g, bnE_b, bnN_g, bnN_b,
                 mlp_U_w, mlp_U_b, mlp_V_w, mlp_V_b):
    import ml_dtypes
    f32 = np.float32
    bf16 = ml_dtypes.bfloat16
    lens = mask.sum(axis=1).astype(np.int64)

    # packed weight matrix
    wm = np.zeros((128, 3 * 512 + 128 + 2), f32)
    for l in range(L):
        wm[:, l * 512:l * 512 + 128] = eU_w[l]
        wm[:, l * 512 + 128:l * 512 + 256] = eV_w[l]
        wm[:, l * 512 + 256:l * 512 + 384] = nU_w[l]
        wm[:, l * 512 + 384:l * 512 + 512] = nV_w[l]
    wm[:, 3 * 512:3 * 512 + 128] = mlp_U_w
    wm[:, 3 * 512 + 128:3 * 512 + 130] = mlp_V_w

    vb = np.zeros((128, 7 * L + 1), f32)
    for l in range(L):
        vb[:, 7 * l + 0] = eV_b[l]
        vb[:, 7 * l + 1] = nU_b[l]
        vb[:, 7 * l + 2] = nV_b[l]
        vb[:, 7 * l + 3] = bnE_g[l]
        vb[:, 7 * l + 4] = bnE_b[l]
        vb[:, 7 * l + 5] = bnN_g[l]
        vb[:, 7 * l + 6] = bnN_b[l]
    vb[:, 7 * L] = mlp_U_b

    vf = np.zeros((RPC, 2 * L * 128), f32)
    for l in range(L):
        vf[:, (2 * l) * 128:(2 * l + 1) * 128] = eV_b[l][None, :]
        vf[:, (2 * l + 1) * 128:(2 * l + 2) * 128] = eU_b[l][None, :]

    # embedding expansion matrix: rows {val, 1, t, t^2}
    w0 = np.zeros((4, 128), f32)
    w0[0, :HH] = w_eval[0]
    e0, e1, e2 = emb_edge[0], emb_edge[1], emb_edge[2]
    c2 = (e2 - 2 * e1 + e0) / 2.0
    c1 = (e1 - e0) - c2
    w0[1, HH:] = e0
    w0[2, HH:] = c1
    w0[3, HH:] = c2

    i25 = np.eye(RPC, dtype=f32).astype(bf16)
    ones1 = np.ones((1, 512), f32).astype(bf16)
    mvb = mlp_V_b.reshape(1, 2).astype(f32)
    wco = w_coord.astype(f32)

    tags = x_edges.astype(f32)
    vals = x_edges_values.astype(f32)

    in_maps = []
    for c in range(NCORES):
        rsl = slice(RPC * c, RPC * (c + 1))
        f4 = np.zeros((4, COLS), f32)
        mi = np.zeros((128, ROWS), f32)
        mic = np.zeros((RPC, B), f32)
        xco = np.zeros((2, ROWS), f32)
        xcf = np.zeros((2, B * N), f32)
        for b in range(B):
            v = vals[b, rsl, :].reshape(-1)
            t = tags[b, rsl, :].reshape(-1)
            cs = slice(b * RPC * N, (b + 1) * RPC * N)
            f4[0, cs] = v
            f4[1, cs] = 1.0
            f4[2, cs] = t
            f4[3, cs] = t * t
            mi[:, b * RPC:(b + 1) * RPC] = mask[b, rsl][None, :]
            mic[:, b] = mask[b, rsl]
            xco[:, b * RPC:(b + 1) * RPC] = x_nodes_coord[b, rsl, :].T
            xcf[:, b * N:(b + 1) * N] = x_nodes_coord[b].T
        in_maps.append(dict(
            f4=f4, xcf=xcf, xco=xco, mi=mi, mic=mic, wm=wm, vb=vb, vf=vf,
            i25=i25, ones1=ones1, mvb=mvb, w0=w0, wco=wco,
        ))
    return lens, in_maps


def _assemble(results):
    y = np.zeros((B, N, N, 2), np.float32)
    for c in range(NCORES):
        yc = results[c]["y"].reshape(2, B, RPC, N)
        for b in range(B):
            y[b, RPC * c:RPC * (c + 1), :, 0] = yc[0, b]
            y[b, RPC * c:RPC * (c + 1), :, 1] = yc[1, b]
    return y


def _run(inputs, trace=False):
    from concourse.bass_utils import run_bass_kernel_spmd
    inputs = {k: np.asarray(v) for k, v in inputs.items()}
    lens, in_maps = _prep_inputs(**inputs)
    key = tuple(int(x) for x in lens)
    if key not in _prog_cache:
        cntM = float(sum(x * x for x in key))
        cntX = float(sum(key))
        _prog_cache[key] = _build(list(key), cntM, cntX)
    nc = _prog_cache[key]
    res = run_bass_kernel_spmd(nc, in_maps, core_ids=list(range(NCORES)),
                               trace=trace)
    return _assemble(res.results), res


def kernel(**inputs):
    out, _ = _run(inputs, trace=False)
    return out


def run_traced(inputs):
    import time
    out, res = _run(inputs, trace=False)      # warm (compile + cache)
    t0 = time.perf_counter()
    out, res = _run(inputs, trace=False)
    wall_ns = (time.perf_counter() - t0) * 1e9
    ns = res.exec_time_ns if res.exec_time_ns is not None else int(wall_ns)
    return out, ns
